# revision 12
# baseline (speedup 1.0000x reference)
"""Multi-head causal attention (B=2, S=2048, D=1024, H=16) on 8 TRN2 cores.

Sharding: tensor-parallel over heads. Core c owns heads {2c, 2c+1} and rows
[128c, 128c+128) of Wo. Each core computes its heads' attention and the
partial output projection; the host sums the 8 partials (the "all-reduce")
and adds the bias.

Device layout (all bf16 in SBUF, f32 PSUM accumulation):
  xT      [1024, 4096]  x transposed: xT[d, b*2048+s] = x[b,s,d]
  wq/wk/wv [1024, 128]  two heads' weights packed on columns
  wo      [128, 1024]   Wo rows for this core
  out_pT  [1024, 4096]  partial^T: out_pT[d, b*2048+s]

Per core:
  1. Q^T/K^T/V^T = (w.T @ xT) via PE, d-chunk outer so compute overlaps DMA-in.
  2. V^T transposed back to V[t, k] per (b,h) via PE transposes; ones column
     appended so the attention matmul also produces softmax denominators.
  3. Flash-style causal attention in scores^T orientation: for each key block
     kj (128 rows of t), scores^T[t, s] for all valid s >= 128*kj, exp on ACT
     (no max subtraction needed: |scores*scale| < ~1), triangular mask on the
     diagonal 128x128, then o^T[k, s] accumulated in PSUM over kj.
  4. Normalize by the denominator row (reciprocal + K=1 broadcast matmul).
  5. Partial projection: out_pT = wo_rows.T @ O^T_stack.
"""

import numpy as np
import ml_dtypes

B, S, D, H = 2, 2048, 1024, 16
HD = 64          # head dim
NCORES = 8
HL = H // NCORES  # local heads per core = 2
BS = B * S        # 4096
SCALE = float(D) ** -0.5

BF16 = ml_dtypes.bfloat16

_CACHE = {}


def _build_kernel():
    import concourse.mybir as mybir
    import concourse.tile as tile
    from concourse import bacc

    bf16 = mybir.dt.bfloat16
    f32 = mybir.dt.float32
    Exp = mybir.ActivationFunctionType.Exp

    nc = bacc.Bacc("TRN2", debug=False, enable_asserts=False)
    xT_d = nc.dram_tensor("xT", [D, BS], bf16, kind="ExternalInput").ap()
    wq_d = nc.dram_tensor("wq", [D, 128], bf16, kind="ExternalInput").ap()
    wk_d = nc.dram_tensor("wk", [D, 128], bf16, kind="ExternalInput").ap()
    wv_d = nc.dram_tensor("wv", [D, 128], bf16, kind="ExternalInput").ap()
    wo_d = nc.dram_tensor("wo", [128, D], bf16, kind="ExternalInput").ap()
    # consts: cols 0:128 = upper-tri mask (1 where col >= row), cols 128:192 =
    # 64x64 identity replicated in both partition halves.
    consts_d = nc.dram_tensor("consts", [128, 192], bf16, kind="ExternalInput").ap()
    out_d = nc.dram_tensor("out_pT", [D, BS], bf16, kind="ExternalOutput").ap()

    DC = D // 128   # 8 d-chunks
    NT = S // 128   # 16 key blocks per sequence

    with tile.TileContext(nc) as tc:
        with tc.tile_pool(name="persist", bufs=1) as pp:
            xT = pp.tile([128, DC, BS], bf16, tag="xT")
            qT = pp.tile([128, BS], bf16, tag="qT")
            kT = pp.tile([128, BS], bf16, tag="kT")
            # V in [t, k] layout + ones column: V_sb[:, g, j, 0:64] = V block
            # for s-block g (g = 16*b + t16) and local head j; [..., 64] = 1.0
            V_sb = pp.tile([128, BS // 128, HL, HD + 1], bf16, tag="V")
            OT = pp.tile([128, BS], bf16, tag="OT")
            wq = pp.tile([128, DC, 128], bf16, tag="wq")
            wk = pp.tile([128, DC, 128], bf16, tag="wk")
            wv = pp.tile([128, DC, 128], bf16, tag="wv")
            wo = pp.tile([128, D], bf16, tag="wo")
            consts = pp.tile([128, 192], bf16, tag="consts")
            trimask = consts[:, 0:128]
            # 64x64 identity replicated in both partition halves, so the
            # V-transpose matmul sees lhsT and identity at the same base
            # partition for either local head.
            ident = consts[:, 128:192]
            ones64 = pp.tile([1, 64], f32, tag="ones64")

            # Constants. No gpsimd producers anywhere (a third producer engine
            # pushes consumers past the per-instruction sync-wait limit).
            nc.sync.dma_start(consts[:], consts_d[:])
            nc.vector.memset(ones64[:], 1.0)
            nc.vector.memset(V_sb[:, :, :, HD : HD + 1], 1.0)

            # DMA in. Weights go first: the xT-chunk DMAs behind them on the
            # same HW queues make phase-1's queue waits imply the weight
            # loads, so later consumers (e.g. the first V matmul) don't need
            # an extra DMA wait (matmul carries at most 2 sync waits).
            for w_sb, w_dr in ((wq, wq_d), (wk, wk_d), (wv, wv_d)):
                nc.sync.dma_start(w_sb[:], w_dr.rearrange("(o p) c -> p o c", p=128))
            nc.sync.dma_start(wo[:], wo_d[:])
            xT_r = xT_d.rearrange("(o p) s -> p o s", p=128)
            for o in range(DC):
                nc.sync.dma_start(xT[:, o, :], xT_r[:, o, :])

            # ---- Phase 1: Q^T / K^T projections ----
            with tc.tile_pool(name="ph1psum", bufs=8, space="PSUM") as ph1:
                for w_sb, dst in ((wq, qT), (wk, kT)):
                    ps = [ph1.tile([128, 512], f32, tag="ph1", name=f"ph1_{s}") for s in range(8)]
                    for o in range(DC):
                        for s in range(8):
                            nc.tensor.matmul(
                                ps[s][:],
                                lhsT=w_sb[:, o, :],
                                rhs=xT[:, o, 512 * s : 512 * (s + 1)],
                                start=(o == 0),
                                stop=(o == DC - 1),
                            )
                    for s in range(8):
                        nc.vector.tensor_copy(dst[:, 512 * s : 512 * (s + 1)], ps[s][:])

            # ---- Phase 2: V directly in [t, k] layout ----
            # lhsT = xT s-block (stationary), rhs = wv: psum[s, c] = V block
            # for both local heads side by side.
            with tc.tile_pool(name="vpsum", bufs=4, space="PSUM") as vps:
                for g in range(BS // 128):
                    pv = vps.tile([128, 128], f32, tag="pv", name="pv")
                    for o in range(DC):
                        nc.tensor.matmul(
                            pv[:],
                            lhsT=xT[:, o, 128 * g : 128 * (g + 1)],
                            rhs=wv[:, o, :],
                            start=(o == 0),
                            stop=(o == DC - 1),
                        )
                    nc.vector.tensor_copy(
                        V_sb[:, g, :, 0:HD],
                        pv[:].rearrange("p (j k) -> p j k", j=HL),
                    )

            # ---- Phase 3: causal attention per (b, local head) ----
            with (
                tc.tile_pool(name="po", bufs=2, space="PSUM") as po_pool,
                tc.tile_pool(name="ps", bufs=2, space="PSUM") as ps_pool,
                tc.tile_pool(name="expp", bufs=3) as exp_pool,
                tc.tile_pool(name="recip", bufs=2) as rc_pool,
            ):
                for bh in range(B * HL):
                    b, j = bh // HL, bh % HL
                    qTh = qT[64 * j : 64 * (j + 1), S * b : S * (b + 1)]
                    kTh = kT[64 * j : 64 * (j + 1), S * b : S * (b + 1)]
                    po = [po_pool.tile([HD + 1, 1024], f32, tag="po", name=f"po_{h}") for h in range(2)]
                    for kj in range(NT):
                        s_lo = 128 * kj
                        pieces = ([(s_lo, 1024), (1024, 2048)] if s_lo < 1024
                                  else [(s_lo, 2048)])
                        for p0, p1 in pieces:
                            w = p1 - p0
                            ps = ps_pool.tile([128, 1024], f32, tag="ps", name="ps")
                            for c0 in range(0, w, 512):
                                c1 = min(c0 + 512, w)
                                nc.tensor.matmul(
                                    ps[:, c0:c1],
                                    lhsT=kTh[:, s_lo : s_lo + 128],
                                    rhs=qTh[:, p0 + c0 : p0 + c1],
                                    start=True,
                                    stop=True,
                                )
                            et = exp_pool.tile([128, 1024], bf16, tag="expT", name="et")
                            nc.scalar.activation(et[:, 0:w], ps[:, 0:w], Exp, scale=SCALE)
                            if p0 == s_lo:
                                # diagonal 128x128: keep only s' >= t
                                nc.vector.tensor_mul(
                                    et[:, 0:128], et[:, 0:128], trimask[:]
                                )
                            h = p0 // 1024
                            g0, g1 = p0 - 1024 * h, p1 - 1024 * h
                            # PSUM accumulation groups are tracked per 2KB
                            # bank: start on the bank's first write (kj==0),
                            # stop on its last (bank q of half h is last
                            # written at kj == 8h + 4q + 3).
                            a = g0
                            while a < g1:
                                bnd = min((a // 512 + 1) * 512, g1)
                                q = a // 512
                                nc.tensor.matmul(
                                    po[h][:, a:bnd],
                                    lhsT=V_sb[:, NT * b + kj, j, :],
                                    rhs=et[:, a - g0 : bnd - g0],
                                    start=(kj == 0),
                                    stop=(kj == 8 * h + 4 * q + 3),
                                )
                                a = bnd
                    # normalize: o^T[k, s] / denom[s]
                    for h in range(2):
                        rc = rc_pool.tile([1, 1024], f32, tag="rc", name="rc")
                        nc.vector.reciprocal(rc[:], po[h][HD : HD + 1, :])
                        pb = ps_pool.tile([64, 1024], f32, tag="ps", name="pb")
                        for c in (0, 512):
                            nc.tensor.matmul(
                                pb[:, c : c + 512],
                                lhsT=ones64[:],
                                rhs=rc[:, c : c + 512],
                                start=True,
                                stop=True,
                            )
                        # DVE can read at most one PSUM operand: stage the
                        # broadcast reciprocal in SBUF before the multiply.
                        sb_b = rc_pool.tile([64, 1024], f32, tag="sbb", name="sb_b")
                        nc.vector.tensor_copy(sb_b[:], pb[:])
                        nc.vector.tensor_mul(
                            OT[64 * j : 64 * (j + 1),
                               S * b + 1024 * h : S * b + 1024 * (h + 1)],
                            po[h][0:HD, :],
                            sb_b[:],
                        )

            # ---- Phase 4: partial output projection out_pT = wo.T @ OT ----
            with (
                tc.tile_pool(name="ph4psum", bufs=4, space="PSUM") as ph4,
                tc.tile_pool(name="ph4out", bufs=4) as ph4o,
            ):
                for dc in range(DC):
                    for nb in range(BS // 512):
                        pp4 = ph4.tile([128, 512], f32, tag="p4", name="pp4")
                        nc.tensor.matmul(
                            pp4[:],
                            lhsT=wo[:, 128 * dc : 128 * (dc + 1)],
                            rhs=OT[:, 512 * nb : 512 * (nb + 1)],
                            start=True,
                            stop=True,
                        )
                        ob = ph4o.tile([128, 512], bf16, tag="o4", name="ob")
                        nc.vector.tensor_copy(ob[:], pp4[:])
                        nc.sync.dma_start(
                            out_d[128 * dc : 128 * (dc + 1), 512 * nb : 512 * (nb + 1)],
                            ob[:],
                        )
    nc.compile()
    return nc


def get_nc():
    if "nc" not in _CACHE:
        _CACHE["nc"] = _build_kernel()
    return _CACHE["nc"]


def make_in_maps(x, Wq, Wk, Wv, Wo):
    """Host-side sharding: per-core input dict (numpy, bf16)."""
    x = np.asarray(x, np.float32)
    Wq = np.asarray(Wq, np.float32)
    Wk = np.asarray(Wk, np.float32)
    Wv = np.asarray(Wv, np.float32)
    Wo = np.asarray(Wo, np.float32)
    xT = np.ascontiguousarray(x.transpose(2, 0, 1).reshape(D, BS)).astype(BF16)
    in_maps = []
    for c in range(NCORES):
        h0 = HL * c

        def pack(W):
            # [HL, D, HD] -> [D, HL*HD]
            return np.ascontiguousarray(
                W[h0 : h0 + HL].transpose(1, 0, 2).reshape(D, HL * HD)
            ).astype(BF16)

        in_maps.append(
            {
                "xT": xT,
                "wq": pack(Wq),
                "wk": pack(Wk),
                "wv": pack(Wv),
                "wo": np.ascontiguousarray(Wo[128 * c : 128 * (c + 1), :]).astype(BF16),
                "consts": _make_consts(),
            }
        )
    return in_maps


def _make_consts():
    if "consts" not in _CACHE:
        tri = (np.arange(128)[None, :] >= np.arange(128)[:, None]).astype(np.float32)
        eye = np.eye(64, dtype=np.float32)
        c = np.zeros((128, 192), np.float32)
        c[:, 0:128] = tri
        c[0:64, 128:192] = eye
        c[64:128, 128:192] = eye
        _CACHE["consts"] = c.astype(BF16)
    return _CACHE["consts"]


def combine_partials(partials, bo):
    acc = np.zeros((D, BS), np.float32)
    for p in partials:
        acc += np.asarray(p, np.float32)
    out = acc.reshape(D, B, S).transpose(1, 2, 0) + np.asarray(bo, np.float32)[None, None, :]
    return np.ascontiguousarray(out.astype(np.float32))


def kernel(x, Wq, Wk, Wv, Wo, bo):
    from concourse.bass_utils import run_bass_kernel_spmd

    nc = get_nc()
    in_maps = make_in_maps(x, Wq, Wk, Wv, Wo)
    res = run_bass_kernel_spmd(nc, in_maps, core_ids=list(range(NCORES)))
    partials = [r["out_pT"] for r in res.results]
    return combine_partials(partials, bo)
